# revision 20
# baseline (speedup 1.0000x reference)
"""Trainium2 Bass kernel for multi-head causal self-attention.

Problem: X [4, 2048, 1024] fp32, Wq/Wk/Wv/Wo [1024, 1024], H=16 heads, HD=64.
reference: out = softmax_causal((X@Wq) (X@Wk)^T / 8) (X@Wv) merged @ Wo.

Sharding over 8 NeuronCores: core c handles batch b = c // 2 and head group
hg = c % 2 (8 heads each). Each core computes a partial [2048, 1024] output
(its heads' contribution through Wo's row shard); the host sums the two
partials per batch (the tensor-parallel all-reduce, done during unsharding).

Per-core dataflow (bf16 operands, fp32 PSUM accumulation), software-pipelined
so the PE never starves (keeps the HAM clock-gate warm):

  ramp     X^T via DMA-transpose on BOTH HWDGE rings (sync + scalar);
           Q^T/K^T/V projections for seq chunk 0 pipelined per d-chunk.
  stage j  attention for q-chunk j (512 q rows x all k-blocks <= diag):
             S^T pair [128k, 2x512q] psum (2 banks): both heads' QK^T
               matmuls emitted adjacently with tile_position row packing so
               they run CONCURRENTLY in the PE array (64-contraction each).
             exp on ACT as ONE [128, 2, 512-rs] instruction per k-block
               (both heads), bf16 out; fully-masked leading cols skipped,
               diagonal blocks get a cmask add (DVE) pre-exp.
             AV accumulated over k-blocks into [72, 512] psum per head;
               col 64 of V = ones => row 64 = softmax denominators.
           Interleaved as PE filler: projections for chunk j+1 (stages 0-2)
           and the output projection for chunks 0..2 (stage 3), so the PE
           stream stays dense while ACT works through the exps.
  norm     reciprocal_approx_fast on the denominators (5x faster than the
           iterative divide), gpsimd partition_broadcast, DVE multiply.
  out      OUT [128s, 512c] = O^T.T @ Wo accumulated over 4 head-pair
           chunks; last chunk's final head-pair contribution added
           separately so the tail doesn't serialize.
"""

import itertools
import sys

for _p in ("/opt/trn_rl_repo", "/root/.axon_site/_ro/trn_rl_repo"):
    if _p not in sys.path:
        sys.path.insert(0, _p)

import ml_dtypes
import numpy as np

import concourse.bass as bass
import concourse.mybir as mybir
import concourse.tile as tile
from concourse import bacc
from concourse.bass_utils import run_bass_kernel_spmd

F32 = mybir.dt.float32
BF16 = mybir.dt.bfloat16
EXPF = mybir.ActivationFunctionType.Exp

B, S, D, H = 4, 2048, 1024, 16
HD = D // H           # 64
HL = H // 2           # 8 heads per core
DL = HL * HD          # 512 local proj width
NEG = -30000.0        # causal mask additive value (exp underflows to 0)
VW = 72               # AV lhsT width: 64 V cols + ones col + 7 pad


class _Filler:
    """Interleave a generator of PE work quanta at a fractional rate."""

    def __init__(self, gens):
        self.it = itertools.chain(*gens)
        self.frac = 0.0
        self.done = False

    def pump(self, amount):
        if self.done:
            return
        self.frac += amount
        while self.frac >= 1.0:
            try:
                next(self.it)
            except StopIteration:
                self.done = True
                return
            self.frac -= 1.0

    def drain(self):
        for _ in self.it:
            pass
        self.done = True


def build_program(s=S, d=D, hl=HL):
    dl = hl * HD
    n_st = s // 128          # 16 s-tiles (128 rows)
    n_dc = d // 128          # 8 d-chunks (projection contraction)
    n_pc = dl // 128         # 4 head-pair chunks
    n_q = s // 512           # 4 q-chunks
    n_k = s // 128           # 16 k-blocks
    n_cc = d // 512          # 2 out column chunks

    nc = bacc.Bacc("TRN2", target_bir_lowering=False, debug=False)

    X = nc.dram_tensor("X", [s, d], BF16, kind="ExternalInput")
    WQ = nc.dram_tensor("WQ", [d, dl], BF16, kind="ExternalInput")
    WK = nc.dram_tensor("WK", [d, dl], BF16, kind="ExternalInput")
    WV = nc.dram_tensor("WV", [d, dl], BF16, kind="ExternalInput")
    WO = nc.dram_tensor("WO", [dl, d], BF16, kind="ExternalInput")
    OUT = nc.dram_tensor("OUT", [s, d], BF16, kind="ExternalOutput")
    # last head-pair's contribution to the last seq chunk, summed on host
    # (avoids serializing the tail on an on-chip add)
    OUT2 = nc.dram_tensor("OUT2", [512, d], BF16, kind="ExternalOutput")

    with tile.TileContext(nc) as tc:
        with tc.tile_pool(name="persist", bufs=1) as persist:
            # diagonal causal mask block x2 (keep where q >= k), one copy
            # per head so a single DVE add masks both heads' diag blocks
            cmask2 = persist.tile([128, 2, 128], F32, name="cmask2")
            nc.gpsimd.memset(cmask2[:], 0.0)
            for hb in (0, 1):
                nc.gpsimd.affine_select(
                    out=cmask2[:, hb, :], in_=cmask2[:, hb, :],
                    compare_op=mybir.AluOpType.is_ge, fill=NEG,
                    base=0, pattern=[[1, 128]], channel_multiplier=-1,
                )

            # X^T in chunk-major layout: xt[p, nq, dc, m] = X^T[dc*128+p,
            # nq*512+m]. Each seq-quarter of X is one CONTIGUOUS DMA
            # transpose writing one contiguous SBUF region — DMA transposes
            # serialize globally against all other DMAs (HW deadlock guard),
            # so fewer/bigger transposes shorten the ramp chain.
            xt = persist.tile([128, n_q, n_dc, 512], BF16, name="xt")
            qt = [persist.tile([128, s], BF16, name=f"qt{i}") for i in range(n_pc)]
            kt = [persist.tile([128, s], BF16, name=f"kt{i}") for i in range(n_pc)]
            vt = [persist.tile([128, hl, VW], BF16, name=f"vt{i}") for i in range(n_st)]
            ot = [persist.tile([128, s], BF16, name=f"ot{i}") for i in range(n_pc)]
            wq = persist.tile([128, n_dc, dl], BF16, name="wq")
            wk = persist.tile([128, n_dc, dl], BF16, name="wk")
            wv = persist.tile([128, n_dc, dl], BF16, name="wv")
            wo = persist.tile([128, n_pc, d], BF16, name="wo")

            # DMA kickoff: ALL on the sync ring in dependency order. DMA
            # transposes serialize globally against in-flight DMAs (HW
            # deadlock guard); every plain-DMA/transpose alternation pays
            # a multi-us completion-latency hop, so weights go first in
            # one batch, then the four transposes back-to-back.
            nc.sync.dma_start(
                wq[:], WQ.ap().rearrange("(c p) m -> p c m", p=128))
            nc.sync.dma_start(
                wk[:], WK.ap().rearrange("(c p) m -> p c m", p=128))
            nc.sync.dma_start(
                wv[:], WV.ap().rearrange("(c p) m -> p c m", p=128))
            nc.sync.dma_start(
                wo[:], WO.ap().rearrange("(c p) m -> p c m", p=128))
            for nq in range(n_q):
                nc.sync.dma_start(
                    xt[:, nq], X[nq * 512:(nq + 1) * 512, :], transpose=True)

            with (
                tc.tile_pool(name="ppp", bufs=2, space="PSUM") as ppp,
                tc.tile_pool(name="stpp", bufs=2, space="PSUM") as stpp,
                tc.tile_pool(name="avp", bufs=2, space="PSUM") as avp,
                tc.tile_pool(name="work", bufs=4) as work,
                tc.tile_pool(name="osbp", bufs=8) as osbp,
            ):
                def gen_proj(nq):
                    """Projection of seq chunk nq; yields per PE quantum.
                    Order Q-pc, K-pc, V-st interleaved so the first
                    attention unit's inputs land earliest."""
                    for pc in range(n_pc):
                        for w, dst, cpy in ((wq, qt, "act"), (wk, kt, "dve")):
                            ps = ppp.tile([128, 512], F32, tag="pp",
                                          name=f"psp{nq}_{pc}")
                            for dc in range(n_dc):
                                nc.tensor.matmul(
                                    ps[:], w[:, dc, pc * 128:(pc + 1) * 128],
                                    xt[:, nq, dc, :],
                                    start=(dc == 0), stop=(dc == n_dc - 1))
                                yield
                            qs = slice(nq * 512, (nq + 1) * 512)
                            if cpy == "act":
                                nc.scalar.copy(dst[pc][:, qs], ps[:])
                            else:
                                nc.vector.tensor_copy(dst[pc][:, qs], ps[:])
                            yield
                        st = 4 * nq + pc
                        ps = ppp.tile([128, dl], F32, tag="pp",
                                      name=f"psv{nq}_{st}")
                        for dc in range(n_dc):
                            nc.tensor.matmul(
                                ps[:], xt[:, nq, dc, pc * 128:(pc + 1) * 128],
                                wv[:, dc, :],
                                start=(dc == 0), stop=(dc == n_dc - 1))
                            yield
                        nc.gpsimd.memset(vt[st][:], 1.0)
                        nc.vector.tensor_copy(
                            vt[st][:, :, 0:64],
                            ps[:].rearrange("p (h e) -> p h e", h=hl))
                        yield

                def gen_outproj(j, skip_last_pc=False):
                    """Output projection for seq chunk j. With skip_last_pc,
                    only head-pairs 0..n_pc-2 are accumulated and written to
                    OUT; the last pair goes to OUT2 via the finisher once
                    its normalize lands (summed on the host)."""
                    npc = n_pc - 1 if skip_last_pc else n_pc
                    for st in range(4 * j, 4 * j + 4):
                        for cc in range(n_cc):
                            ps = ppp.tile([128, 512], F32, tag="pp",
                                          name=f"pso{st}_{cc}")
                            for pc in range(npc):
                                nc.tensor.matmul(
                                    ps[:], ot[pc][:, st * 128:(st + 1) * 128],
                                    wo[:, pc, cc * 512:(cc + 1) * 512],
                                    start=(pc == 0), stop=(pc == npc - 1))
                                yield
                            osb = osbp.tile([128, 512], BF16, tag="osb",
                                            name=f"osb{st}_{cc}")
                            nc.vector.tensor_copy(osb[:], ps[:])
                            yield
                            # alternate rings so OUT writes don't back up
                            eng = nc.sync if (st + cc) % 2 == 0 else nc.scalar
                            eng.dma_start(
                                OUT[st * 128:(st + 1) * 128,
                                    cc * 512:(cc + 1) * 512],
                                osb[:])
                            yield

                def attn_unit(j, pc, fillers):
                    js = slice(j * 512, (j + 1) * 512)
                    n_i = min(4 * j + 4, n_k)
                    av = [avp.tile([VW, 512], F32, tag="av",
                                   name=f"av{j}_{pc}_{h}") for h in (0, 1)]
                    for i in range(n_i):
                        r = i - 4 * j
                        rs = max(r, 0) * 128   # fully-masked leading cols
                        stp = stpp.tile([128, 2, 512], F32, tag="stp",
                                        name=f"stp{j}_{pc}_{i}")
                        for h in (0, 1):
                            hs = slice(64 * h, 64 * h + 64)
                            nc.tensor.matmul(
                                stp[:, h, rs:512],
                                kt[pc][hs, i * 128:(i + 1) * 128],
                                qt[pc][hs, j * 512 + rs:(j + 1) * 512],
                                start=True, stop=True,
                                tile_position=(64 * h, 0))
                        if r >= 0:
                            nc.vector.tensor_add(
                                stp[:, :, rs:rs + 128], stp[:, :, rs:rs + 128],
                                cmask2[:])
                        et = work.tile([128, 2, 512], BF16, tag="et", bufs=6,
                                       name=f"et{j}_{pc}_{i}")
                        nc.scalar.activation(
                            et[:, :, rs:512], stp[:, :, rs:512], EXPF,
                            scale=0.125)
                        # half the filler between exp and AV: the AV matmuls
                        # wait on exp completion (~0.6us) — give the PE
                        # ready work at exactly that point in priority order
                        for flr, rate in fillers:
                            flr.pump(rate / 2)
                        for h in (0, 1):
                            nc.tensor.matmul(
                                av[h][:, rs:512], vt[i][:, 2 * pc + h, :],
                                et[:, h, rs:512],
                                start=(i == 0), stop=(i == n_i - 1))
                        for flr, rate in fillers:
                            flr.pump(rate / 2)

                    # normalize: denominators live in av row 64. Per-head
                    # independent chains: copy O'+denom to SBUF, DMA the
                    # denom row to partition 0, approx-reciprocal, gpsimd
                    # broadcast, DVE multiply (h1 DMA-shifts to rows 64-127).
                    for h in (0, 1):
                        orw = work.tile([VW, 512], F32, tag="orw", bufs=4,
                                        name=f"orw{j}_{pc}_{h}")
                        nc.vector.tensor_copy(orw[:], av[h][:])
                        dgp = work.tile([1, 512], F32, tag=f"dg{h}", bufs=3,
                                        name=f"dg{j}_{pc}_{h}")
                        nc.sync.dma_start(dgp[:], orw[64:65, :])
                        rgp = work.tile([1, 512], F32, tag=f"rg{h}", bufs=3,
                                        name=f"rg{j}_{pc}_{h}")
                        nc.vector.reciprocal_approx_fast(rgp[:], dgp[:])
                        bc = work.tile([64, 512], F32, tag=f"bc{h}", bufs=3,
                                       name=f"bc{j}_{pc}_{h}")
                        nc.gpsimd.partition_broadcast(bc[:], rgp[:])
                        if h == 0:
                            nc.vector.tensor_mul(
                                ot[pc][0:64, js], orw[0:64, :], bc[:])
                        else:
                            sc = work.tile([64, 512], BF16, tag="sc", bufs=3,
                                           name=f"sc{j}_{pc}")
                            nc.vector.tensor_mul(sc[:], orw[0:64, :], bc[:])
                            nc.sync.dma_start(ot[pc][64:128, js], sc[:])

                # ---- PE warmup: dummy matmuls during the DMA ramp keep
                # the HAM activity window busy so the first real matmuls
                # run at 2.4 GHz instead of the cold 1.2 GHz ----
                warm = ppp.tile([64, 128], F32, tag="pp", name="warm")
                for wmm in range(48):
                    nc.tensor.matmul(
                        warm[:], cmask2[:, 0, 0:64], cmask2[:, 0, :],
                        start=(wmm == 0), stop=(wmm == 47))

                # ---- ramp: projections for chunk 0 ----
                for _ in gen_proj(0):
                    pass

                # ---- pipelined stages ----
                for j in range(n_q):
                    if j < n_q - 1:
                        filler = _Filler([gen_proj(j + 1)])
                        rate = {0: 7.0, 1: 3.6, 2: 2.4}[j]
                    else:
                        # outproj(2) is held back past the last unit so the
                        # PE has work while the final normalize drains
                        filler = _Filler([gen_outproj(0), gen_outproj(1)])
                        rate = 1.6
                    for pc in range(n_pc):
                        fillers = [(filler, rate)]
                        if j == n_q - 1 and pc == n_pc - 1:
                            part1 = _Filler([gen_outproj(3, skip_last_pc=True)])
                            fillers.append((part1, 2.0))
                        attn_unit(j, pc, fillers)
                        if j == n_q - 1 and pc == n_pc - 1:
                            part1.drain()
                    filler.drain()
                for _ in gen_outproj(2):
                    pass

                # ---- finisher: last head-pair x chunk 3 -> OUT2 ----
                for st in range(4 * (n_q - 1), 4 * n_q):
                    for cc in range(n_cc):
                        psb = ppp.tile([128, 512], F32, tag="pp",
                                       name=f"psb{st}_{cc}")
                        nc.tensor.matmul(
                            psb[:], ot[n_pc - 1][:, st * 128:(st + 1) * 128],
                            wo[:, n_pc - 1, cc * 512:(cc + 1) * 512],
                            start=True, stop=True)
                        osb = osbp.tile([128, 512], BF16, tag="osb",
                                        name=f"osb2{st}_{cc}")
                        nc.vector.tensor_copy(osb[:], psb[:])
                        eng = nc.sync if (st + cc) % 2 == 0 else nc.scalar
                        eng.dma_start(
                            OUT2[(st - 4 * (n_q - 1)) * 128:
                                 (st - 4 * (n_q - 1) + 1) * 128,
                                 cc * 512:(cc + 1) * 512],
                            osb[:])

    nc.compile()
    return nc


_NC_CACHE = {}


def _get_program():
    key = (S, D, HL)
    if key not in _NC_CACHE:
        _NC_CACHE[key] = build_program()
    return _NC_CACHE[key]


def _bf16(a):
    return np.ascontiguousarray(a.astype(ml_dtypes.bfloat16))


def make_in_maps(X, Wq, Wk, Wv, Wo):
    in_maps = []
    for c in range(8):
        b, hg = c // 2, c % 2
        cs = slice(hg * DL, hg * DL + DL)
        in_maps.append({
            "X": _bf16(X[b]),
            "WQ": _bf16(Wq[:, cs]),
            "WK": _bf16(Wk[:, cs]),
            "WV": _bf16(Wv[:, cs]),
            "WO": _bf16(Wo[cs, :]),
        })
    return in_maps


def gather_out(results):
    out = np.empty((B, S, D), dtype=np.float32)
    for b in range(B):
        out[b] = (results[2 * b]["OUT"].astype(np.float32)
                  + results[2 * b + 1]["OUT"].astype(np.float32))
        out[b, S - 512:] += (results[2 * b]["OUT2"].astype(np.float32)
                             + results[2 * b + 1]["OUT2"].astype(np.float32))
    return out


def kernel(X, Wq, Wk, Wv, Wo):
    X = np.asarray(X, dtype=np.float32)
    Wq = np.asarray(Wq, dtype=np.float32)
    Wk = np.asarray(Wk, dtype=np.float32)
    Wv = np.asarray(Wv, dtype=np.float32)
    Wo = np.asarray(Wo, dtype=np.float32)

    nc = _get_program()
    in_maps = make_in_maps(X, Wq, Wk, Wv, Wo)
    res = run_bass_kernel_spmd(nc, in_maps, list(range(8)), trace=False)
    return gather_out(res.results)


if __name__ == "__main__":
    rng = np.random.default_rng(0)
    scale = 1.0 / np.sqrt(D)
    inputs = {
        "X": rng.standard_normal((B, S, D), dtype=np.float32),
        "Wq": rng.standard_normal((D, D), dtype=np.float32) * scale,
        "Wk": rng.standard_normal((D, D), dtype=np.float32) * scale,
        "Wv": rng.standard_normal((D, D), dtype=np.float32) * scale,
        "Wo": rng.standard_normal((D, D), dtype=np.float32) * scale,
    }
    out = kernel(**inputs)
    print("kernel output shape:", out.shape)


# revision 21
# speedup vs baseline: 1.0225x; 1.0225x over previous
"""Trainium2 Bass kernel for multi-head causal self-attention.

Problem: X [4, 2048, 1024] fp32, Wq/Wk/Wv/Wo [1024, 1024], H=16 heads, HD=64.
reference: out = softmax_causal((X@Wq) (X@Wk)^T / 8) (X@Wv) merged @ Wo.

Sharding over 8 NeuronCores: core c handles batch b = c // 2 and head group
hg = c % 2 (8 heads each). Each core computes a partial [2048, 1024] output
(its heads' contribution through Wo's row shard); the host sums the two
partials per batch (the tensor-parallel all-reduce, done during unsharding).

Per-core dataflow (bf16 operands, fp32 PSUM accumulation), software-pipelined
so the PE never starves (keeps the HAM clock-gate warm):

  ramp     weights then four contiguous seq-quarter X^T DMA transposes,
           all on one HWDGE ring in dependency order (transposes serialize
           globally against in-flight DMAs, and each plain/transpose
           alternation costs a multi-us completion-latency hop);
           Q^T/K^T/V projections for seq chunk 0 follow.
  stage j  attention for q-chunk j (512 q rows x all k-blocks <= diag):
             S^T pair [128k, 2x512q] psum (2 banks): both heads' QK^T
               matmuls emitted adjacently with tile_position row packing so
               they run CONCURRENTLY in the PE array (64-contraction each).
             exp on ACT as ONE [128, 2, 512-rs] instruction per k-block
               (both heads), bf16 out; fully-masked leading cols skipped,
               diagonal blocks get a cmask add (DVE) pre-exp.
             AV accumulated over k-blocks into [72, 512] psum per head;
               col 64 of V = ones => row 64 = softmax denominators.
           Interleaved as PE filler: projections for chunk j+1 (stages 0-2)
           and the output projection for chunks 0..2 (stage 3), so the PE
           stream stays dense while ACT works through the exps.
  norm     reciprocal_approx_fast on the denominators (5x faster than the
           iterative divide), gpsimd partition_broadcast, DVE multiply.
  out      OUT [128s, 512c] = O^T.T @ Wo accumulated over 4 head-pair
           chunks, bf16, DMAs alternating both rings; the last chunk's
           final head-pair contribution goes to OUT2 and is added on the
           host so the tail doesn't serialize on an on-chip add.
"""

import itertools
import sys

for _p in ("/opt/trn_rl_repo", "/root/.axon_site/_ro/trn_rl_repo"):
    if _p not in sys.path:
        sys.path.insert(0, _p)

import ml_dtypes
import numpy as np

import concourse.bass as bass
import concourse.mybir as mybir
import concourse.tile as tile
from concourse import bacc
from concourse.bass_utils import run_bass_kernel_spmd

F32 = mybir.dt.float32
BF16 = mybir.dt.bfloat16
EXPF = mybir.ActivationFunctionType.Exp

B, S, D, H = 4, 2048, 1024, 16
HD = D // H           # 64
HL = H // 2           # 8 heads per core
DL = HL * HD          # 512 local proj width
NEG = -30000.0        # causal mask additive value (exp underflows to 0)
VW = 72               # AV lhsT width: 64 V cols + ones col + 7 pad


class _Filler:
    """Interleave a generator of PE work quanta at a fractional rate."""

    def __init__(self, gens):
        self.it = itertools.chain(*gens)
        self.frac = 0.0
        self.done = False

    def pump(self, amount):
        if self.done:
            return
        self.frac += amount
        while self.frac >= 1.0:
            try:
                next(self.it)
            except StopIteration:
                self.done = True
                return
            self.frac -= 1.0

    def drain(self):
        for _ in self.it:
            pass
        self.done = True


def build_program(s=S, d=D, hl=HL):
    dl = hl * HD
    n_st = s // 128          # 16 s-tiles (128 rows)
    n_dc = d // 128          # 8 d-chunks (projection contraction)
    n_pc = dl // 128         # 4 head-pair chunks
    n_q = s // 512           # 4 q-chunks
    n_k = s // 128           # 16 k-blocks
    n_cc = d // 512          # 2 out column chunks

    nc = bacc.Bacc("TRN2", target_bir_lowering=False, debug=False)

    X = nc.dram_tensor("X", [s, d], BF16, kind="ExternalInput")
    WQ = nc.dram_tensor("WQ", [d, dl], BF16, kind="ExternalInput")
    WK = nc.dram_tensor("WK", [d, dl], BF16, kind="ExternalInput")
    WV = nc.dram_tensor("WV", [d, dl], BF16, kind="ExternalInput")
    WO = nc.dram_tensor("WO", [dl, d], BF16, kind="ExternalInput")
    OUT = nc.dram_tensor("OUT", [s, d], BF16, kind="ExternalOutput")
    # last head-pair's contribution to the last seq chunk, summed on host
    # (avoids serializing the tail on an on-chip add)
    OUT2 = nc.dram_tensor("OUT2", [512, d], BF16, kind="ExternalOutput")

    with tile.TileContext(nc) as tc:
        with tc.tile_pool(name="persist", bufs=1) as persist:
            # diagonal causal mask block x2 (keep where q >= k), one copy
            # per head so a single DVE add masks both heads' diag blocks
            cmask2 = persist.tile([128, 2, 128], F32, name="cmask2")
            nc.gpsimd.memset(cmask2[:], 0.0)
            for hb in (0, 1):
                nc.gpsimd.affine_select(
                    out=cmask2[:, hb, :], in_=cmask2[:, hb, :],
                    compare_op=mybir.AluOpType.is_ge, fill=NEG,
                    base=0, pattern=[[1, 128]], channel_multiplier=-1,
                )

            # X^T in chunk-major layout: xt[p, nq, dc, m] = X^T[dc*128+p,
            # nq*512+m]. Each seq-quarter of X is one CONTIGUOUS DMA
            # transpose writing one contiguous SBUF region — DMA transposes
            # serialize globally against all other DMAs (HW deadlock guard),
            # so fewer/bigger transposes shorten the ramp chain.
            xt = persist.tile([128, n_q, n_dc, 512], BF16, name="xt")
            qt = [persist.tile([128, s], BF16, name=f"qt{i}") for i in range(n_pc)]
            kt = [persist.tile([128, s], BF16, name=f"kt{i}") for i in range(n_pc)]
            vt = [persist.tile([128, hl, VW], BF16, name=f"vt{i}") for i in range(n_st)]
            ot = [persist.tile([128, s], BF16, name=f"ot{i}") for i in range(n_pc)]
            wq = persist.tile([128, n_dc, dl], BF16, name="wq")
            wk = persist.tile([128, n_dc, dl], BF16, name="wk")
            wv = persist.tile([128, n_dc, dl], BF16, name="wv")
            wo = persist.tile([128, n_pc, d], BF16, name="wo")

            # DMA kickoff: ALL on the sync ring in dependency order. DMA
            # transposes serialize globally against in-flight DMAs (HW
            # deadlock guard); every plain-DMA/transpose alternation pays
            # a multi-us completion-latency hop, so weights go first in
            # one batch, then the four transposes back-to-back.
            nc.sync.dma_start(
                wq[:], WQ.ap().rearrange("(c p) m -> p c m", p=128))
            nc.sync.dma_start(
                wk[:], WK.ap().rearrange("(c p) m -> p c m", p=128))
            nc.sync.dma_start(
                wv[:], WV.ap().rearrange("(c p) m -> p c m", p=128))
            nc.sync.dma_start(
                wo[:], WO.ap().rearrange("(c p) m -> p c m", p=128))
            for nq in range(n_q):
                nc.sync.dma_start(
                    xt[:, nq], X[nq * 512:(nq + 1) * 512, :], transpose=True)

            with (
                tc.tile_pool(name="ppp", bufs=2, space="PSUM") as ppp,
                tc.tile_pool(name="stpp", bufs=2, space="PSUM") as stpp,
                tc.tile_pool(name="avp", bufs=2, space="PSUM") as avp,
                tc.tile_pool(name="work", bufs=4) as work,
                tc.tile_pool(name="osbp", bufs=8) as osbp,
            ):
                def gen_proj(nq):
                    """Projection of seq chunk nq; yields per PE quantum.
                    Order Q-pc, K-pc, V-st interleaved so the first
                    attention unit's inputs land earliest."""
                    for pc in range(n_pc):
                        for w, dst, cpy in ((wq, qt, "act"), (wk, kt, "dve")):
                            ps = ppp.tile([128, 512], F32, tag="pp",
                                          name=f"psp{nq}_{pc}")
                            for dc in range(n_dc):
                                nc.tensor.matmul(
                                    ps[:], w[:, dc, pc * 128:(pc + 1) * 128],
                                    xt[:, nq, dc, :],
                                    start=(dc == 0), stop=(dc == n_dc - 1))
                                yield
                            qs = slice(nq * 512, (nq + 1) * 512)
                            if cpy == "act":
                                nc.scalar.copy(dst[pc][:, qs], ps[:])
                            else:
                                nc.vector.tensor_copy(dst[pc][:, qs], ps[:])
                            yield
                        st = 4 * nq + pc
                        ps = ppp.tile([128, dl], F32, tag="pp",
                                      name=f"psv{nq}_{st}")
                        for dc in range(n_dc):
                            nc.tensor.matmul(
                                ps[:], xt[:, nq, dc, pc * 128:(pc + 1) * 128],
                                wv[:, dc, :],
                                start=(dc == 0), stop=(dc == n_dc - 1))
                            yield
                        nc.gpsimd.memset(vt[st][:], 1.0)
                        nc.vector.tensor_copy(
                            vt[st][:, :, 0:64],
                            ps[:].rearrange("p (h e) -> p h e", h=hl))
                        yield

                def gen_outproj(j, skip_last_pc=False):
                    """Output projection for seq chunk j. With skip_last_pc,
                    only head-pairs 0..n_pc-2 are accumulated and written to
                    OUT; the last pair goes to OUT2 via the finisher once
                    its normalize lands (summed on the host)."""
                    npc = n_pc - 1 if skip_last_pc else n_pc
                    for st in range(4 * j, 4 * j + 4):
                        for cc in range(n_cc):
                            ps = ppp.tile([128, 512], F32, tag="pp",
                                          name=f"pso{st}_{cc}")
                            for pc in range(npc):
                                nc.tensor.matmul(
                                    ps[:], ot[pc][:, st * 128:(st + 1) * 128],
                                    wo[:, pc, cc * 512:(cc + 1) * 512],
                                    start=(pc == 0), stop=(pc == npc - 1))
                                yield
                            osb = osbp.tile([128, 512], BF16, tag="osb",
                                            name=f"osb{st}_{cc}")
                            nc.vector.tensor_copy(osb[:], ps[:])
                            yield
                            # alternate rings so OUT writes don't back up
                            eng = nc.sync if (st + cc) % 2 == 0 else nc.scalar
                            eng.dma_start(
                                OUT[st * 128:(st + 1) * 128,
                                    cc * 512:(cc + 1) * 512],
                                osb[:])
                            yield

                def attn_unit(j, pc, fillers):
                    js = slice(j * 512, (j + 1) * 512)
                    n_i = min(4 * j + 4, n_k)
                    av = [avp.tile([VW, 512], F32, tag="av",
                                   name=f"av{j}_{pc}_{h}") for h in (0, 1)]
                    for i in range(n_i):
                        r = i - 4 * j
                        rs = max(r, 0) * 128   # fully-masked leading cols
                        stp = stpp.tile([128, 2, 512], F32, tag="stp",
                                        name=f"stp{j}_{pc}_{i}")
                        for h in (0, 1):
                            hs = slice(64 * h, 64 * h + 64)
                            nc.tensor.matmul(
                                stp[:, h, rs:512],
                                kt[pc][hs, i * 128:(i + 1) * 128],
                                qt[pc][hs, j * 512 + rs:(j + 1) * 512],
                                start=True, stop=True,
                                tile_position=(64 * h, 0))
                        if r >= 0:
                            nc.vector.tensor_add(
                                stp[:, :, rs:rs + 128], stp[:, :, rs:rs + 128],
                                cmask2[:])
                        et = work.tile([128, 2, 512], BF16, tag="et", bufs=6,
                                       name=f"et{j}_{pc}_{i}")
                        nc.scalar.activation(
                            et[:, :, rs:512], stp[:, :, rs:512], EXPF,
                            scale=0.125)
                        # half the filler between exp and AV: the AV matmuls
                        # wait on exp completion (~0.6us) — give the PE
                        # ready work at exactly that point in priority order
                        for flr, rate in fillers:
                            flr.pump(rate / 2)
                        for h in (0, 1):
                            nc.tensor.matmul(
                                av[h][:, rs:512], vt[i][:, 2 * pc + h, :],
                                et[:, h, rs:512],
                                start=(i == 0), stop=(i == n_i - 1))
                        for flr, rate in fillers:
                            flr.pump(rate / 2)

                    # normalize: denominators live in av row 64. Per-head
                    # independent chains: copy O'+denom to SBUF, DMA the
                    # denom row to partition 0, approx-reciprocal, gpsimd
                    # broadcast, DVE multiply (h1 DMA-shifts to rows 64-127).
                    for h in (0, 1):
                        orw = work.tile([VW, 512], F32, tag="orw", bufs=4,
                                        name=f"orw{j}_{pc}_{h}")
                        nc.vector.tensor_copy(orw[:], av[h][:])
                        dgp = work.tile([1, 512], F32, tag=f"dg{h}", bufs=3,
                                        name=f"dg{j}_{pc}_{h}")
                        nc.sync.dma_start(dgp[:], orw[64:65, :])
                        rgp = work.tile([1, 512], F32, tag=f"rg{h}", bufs=3,
                                        name=f"rg{j}_{pc}_{h}")
                        nc.vector.reciprocal_approx_fast(rgp[:], dgp[:])
                        bc = work.tile([64, 512], F32, tag=f"bc{h}", bufs=3,
                                       name=f"bc{j}_{pc}_{h}")
                        nc.gpsimd.partition_broadcast(bc[:], rgp[:])
                        if h == 0:
                            nc.vector.tensor_mul(
                                ot[pc][0:64, js], orw[0:64, :], bc[:])
                        else:
                            sc = work.tile([64, 512], BF16, tag="sc", bufs=3,
                                           name=f"sc{j}_{pc}")
                            nc.vector.tensor_mul(sc[:], orw[0:64, :], bc[:])
                            nc.sync.dma_start(ot[pc][64:128, js], sc[:])

                # ---- ramp: projections for chunk 0 ----
                for _ in gen_proj(0):
                    pass

                # ---- pipelined stages ----
                for j in range(n_q):
                    if j < n_q - 1:
                        filler = _Filler([gen_proj(j + 1)])
                        rate = {0: 7.0, 1: 3.6, 2: 2.4}[j]
                    else:
                        filler = _Filler([gen_outproj(0), gen_outproj(1),
                                          gen_outproj(2)])
                        rate = 2.2
                    for pc in range(n_pc):
                        fillers = [(filler, rate)]
                        if j == n_q - 1 and pc == n_pc - 1:
                            part1 = _Filler([gen_outproj(3, skip_last_pc=True)])
                            fillers.append((part1, 2.0))
                        attn_unit(j, pc, fillers)
                        if j == n_q - 1 and pc == n_pc - 1:
                            part1.drain()
                    filler.drain()

                # ---- finisher: last head-pair x chunk 3 -> OUT2 ----
                for st in range(4 * (n_q - 1), 4 * n_q):
                    for cc in range(n_cc):
                        psb = ppp.tile([128, 512], F32, tag="pp",
                                       name=f"psb{st}_{cc}")
                        nc.tensor.matmul(
                            psb[:], ot[n_pc - 1][:, st * 128:(st + 1) * 128],
                            wo[:, n_pc - 1, cc * 512:(cc + 1) * 512],
                            start=True, stop=True)
                        osb = osbp.tile([128, 512], BF16, tag="osb",
                                        name=f"osb2{st}_{cc}")
                        nc.vector.tensor_copy(osb[:], psb[:])
                        eng = nc.sync if (st + cc) % 2 == 0 else nc.scalar
                        eng.dma_start(
                            OUT2[(st - 4 * (n_q - 1)) * 128:
                                 (st - 4 * (n_q - 1) + 1) * 128,
                                 cc * 512:(cc + 1) * 512],
                            osb[:])

    nc.compile()
    return nc


_NC_CACHE = {}


def _get_program():
    key = (S, D, HL)
    if key not in _NC_CACHE:
        _NC_CACHE[key] = build_program()
    return _NC_CACHE[key]


def _bf16(a):
    return np.ascontiguousarray(a.astype(ml_dtypes.bfloat16))


def make_in_maps(X, Wq, Wk, Wv, Wo):
    in_maps = []
    for c in range(8):
        b, hg = c // 2, c % 2
        cs = slice(hg * DL, hg * DL + DL)
        in_maps.append({
            "X": _bf16(X[b]),
            "WQ": _bf16(Wq[:, cs]),
            "WK": _bf16(Wk[:, cs]),
            "WV": _bf16(Wv[:, cs]),
            "WO": _bf16(Wo[cs, :]),
        })
    return in_maps


def gather_out(results):
    out = np.empty((B, S, D), dtype=np.float32)
    for b in range(B):
        out[b] = (results[2 * b]["OUT"].astype(np.float32)
                  + results[2 * b + 1]["OUT"].astype(np.float32))
        out[b, S - 512:] += (results[2 * b]["OUT2"].astype(np.float32)
                             + results[2 * b + 1]["OUT2"].astype(np.float32))
    return out


def kernel(X, Wq, Wk, Wv, Wo):
    X = np.asarray(X, dtype=np.float32)
    Wq = np.asarray(Wq, dtype=np.float32)
    Wk = np.asarray(Wk, dtype=np.float32)
    Wv = np.asarray(Wv, dtype=np.float32)
    Wo = np.asarray(Wo, dtype=np.float32)

    nc = _get_program()
    in_maps = make_in_maps(X, Wq, Wk, Wv, Wo)
    res = run_bass_kernel_spmd(nc, in_maps, list(range(8)), trace=False)
    return gather_out(res.results)


if __name__ == "__main__":
    rng = np.random.default_rng(0)
    scale = 1.0 / np.sqrt(D)
    inputs = {
        "X": rng.standard_normal((B, S, D), dtype=np.float32),
        "Wq": rng.standard_normal((D, D), dtype=np.float32) * scale,
        "Wk": rng.standard_normal((D, D), dtype=np.float32) * scale,
        "Wv": rng.standard_normal((D, D), dtype=np.float32) * scale,
        "Wo": rng.standard_normal((D, D), dtype=np.float32) * scale,
    }
    out = kernel(**inputs)
    print("kernel output shape:", out.shape)


# revision 22
# speedup vs baseline: 1.0280x; 1.0054x over previous
"""Trainium2 Bass kernel for multi-head causal self-attention.

Problem: X [4, 2048, 1024] fp32, Wq/Wk/Wv/Wo [1024, 1024], H=16 heads, HD=64.
reference: out = softmax_causal((X@Wq) (X@Wk)^T / 8) (X@Wv) merged @ Wo.

Sharding over 8 NeuronCores: core c handles batch b = c // 2 and head group
hg = c % 2 (8 heads each). Each core computes a partial [2048, 1024] output
(its heads' contribution through Wo's row shard); the host sums the two
partials per batch (the tensor-parallel all-reduce, done during unsharding).

Per-core dataflow (bf16 operands, fp32 PSUM accumulation), software-pipelined
so the PE never starves (keeps the HAM clock-gate warm):

  ramp     weights then four contiguous seq-quarter X^T DMA transposes,
           all on one HWDGE ring in dependency order (transposes serialize
           globally against in-flight DMAs, and each plain/transpose
           alternation costs a multi-us completion-latency hop);
           Q^T/K^T/V projections for seq chunk 0 follow.
  stage j  attention for q-chunk j (512 q rows x all k-blocks <= diag):
             S^T pair [128k, 2x512q] psum (2 banks): both heads' QK^T
               matmuls emitted adjacently with tile_position row packing so
               they run CONCURRENTLY in the PE array (64-contraction each).
             exp on ACT as ONE [128, 2, 512-rs] instruction per k-block
               (both heads), bf16 out; fully-masked leading cols skipped,
               diagonal blocks get a cmask add (DVE) pre-exp.
             AV accumulated over k-blocks into [72, 512] psum per head;
               col 64 of V = ones => row 64 = softmax denominators.
           Interleaved as PE filler: projections for chunk j+1 (stages 0-2)
           and the output projection for chunks 0..2 (stage 3), so the PE
           stream stays dense while ACT works through the exps.
  norm     reciprocal_approx_fast on the denominators (5x faster than the
           iterative divide), gpsimd partition_broadcast, DVE multiply.
  out      OUT [128s, 512c] = O^T.T @ Wo accumulated over 4 head-pair
           chunks, bf16, DMAs alternating both rings; the last chunk's
           final head-pair contribution goes to OUT2 and is added on the
           host so the tail doesn't serialize on an on-chip add.
"""

import itertools
import sys

for _p in ("/opt/trn_rl_repo", "/root/.axon_site/_ro/trn_rl_repo"):
    if _p not in sys.path:
        sys.path.insert(0, _p)

import ml_dtypes
import numpy as np

import concourse.bass as bass
import concourse.mybir as mybir
import concourse.tile as tile
from concourse import bacc
from concourse.bass_utils import run_bass_kernel_spmd

F32 = mybir.dt.float32
BF16 = mybir.dt.bfloat16
EXPF = mybir.ActivationFunctionType.Exp

B, S, D, H = 4, 2048, 1024, 16
HD = D // H           # 64
HL = H // 2           # 8 heads per core
DL = HL * HD          # 512 local proj width
NEG = -30000.0        # causal mask additive value (exp underflows to 0)
VW = 72               # AV lhsT width: 64 V cols + ones col + 7 pad


class _Filler:
    """Interleave a generator of PE work quanta at a fractional rate."""

    def __init__(self, gens):
        self.it = itertools.chain(*gens)
        self.frac = 0.0
        self.done = False

    def pump(self, amount):
        if self.done:
            return
        self.frac += amount
        while self.frac >= 1.0:
            try:
                next(self.it)
            except StopIteration:
                self.done = True
                return
            self.frac -= 1.0

    def drain(self):
        for _ in self.it:
            pass
        self.done = True


def build_program(s=S, d=D, hl=HL):
    dl = hl * HD
    n_st = s // 128          # 16 s-tiles (128 rows)
    n_dc = d // 128          # 8 d-chunks (projection contraction)
    n_pc = dl // 128         # 4 head-pair chunks
    n_q = s // 512           # 4 q-chunks
    n_k = s // 128           # 16 k-blocks
    n_cc = d // 512          # 2 out column chunks

    nc = bacc.Bacc("TRN2", target_bir_lowering=False, debug=False)

    X = nc.dram_tensor("X", [s, d], BF16, kind="ExternalInput")
    WQ = nc.dram_tensor("WQ", [d, dl], BF16, kind="ExternalInput")
    WK = nc.dram_tensor("WK", [d, dl], BF16, kind="ExternalInput")
    WV = nc.dram_tensor("WV", [d, dl], BF16, kind="ExternalInput")
    WO = nc.dram_tensor("WO", [dl, d], BF16, kind="ExternalInput")
    OUT = nc.dram_tensor("OUT", [s, d], BF16, kind="ExternalOutput")
    # last head-pair's contribution to the last seq chunk, summed on host
    # (avoids serializing the tail on an on-chip add)
    OUT2 = nc.dram_tensor("OUT2", [512, d], BF16, kind="ExternalOutput")

    with tile.TileContext(nc) as tc:
        with tc.tile_pool(name="persist", bufs=1) as persist:
            # diagonal causal mask block x2 (keep where q >= k), one copy
            # per head so a single DVE add masks both heads' diag blocks
            cmask2 = persist.tile([128, 2, 128], F32, name="cmask2")
            nc.gpsimd.memset(cmask2[:], 0.0)
            for hb in (0, 1):
                nc.gpsimd.affine_select(
                    out=cmask2[:, hb, :], in_=cmask2[:, hb, :],
                    compare_op=mybir.AluOpType.is_ge, fill=NEG,
                    base=0, pattern=[[1, 128]], channel_multiplier=-1,
                )

            # X^T in chunk-major layout: xt[p, nq, dc, m] = X^T[dc*128+p,
            # nq*512+m]. Each seq-quarter of X is one CONTIGUOUS DMA
            # transpose writing one contiguous SBUF region — DMA transposes
            # serialize globally against all other DMAs (HW deadlock guard),
            # so fewer/bigger transposes shorten the ramp chain.
            xt = persist.tile([128, n_q, n_dc, 512], BF16, name="xt")
            qt = [persist.tile([128, s], BF16, name=f"qt{i}") for i in range(n_pc)]
            kt = [persist.tile([128, s], BF16, name=f"kt{i}") for i in range(n_pc)]
            vt = [persist.tile([128, hl, VW], BF16, name=f"vt{i}") for i in range(n_st)]
            ot = [persist.tile([128, s], BF16, name=f"ot{i}") for i in range(n_pc)]
            wq = persist.tile([128, n_dc, dl], BF16, name="wq")
            wk = persist.tile([128, n_dc, dl], BF16, name="wk")
            wv = persist.tile([128, n_dc, dl], BF16, name="wv")
            wo = persist.tile([128, n_pc, d], BF16, name="wo")

            # DMA kickoff: ALL on the sync ring in dependency order. DMA
            # transposes serialize globally against in-flight DMAs (HW
            # deadlock guard); every plain-DMA/transpose alternation pays
            # a multi-us completion-latency hop, so weights go first in
            # one batch, then the four transposes back-to-back.
            nc.sync.dma_start(
                wq[:], WQ.ap().rearrange("(c p) m -> p c m", p=128))
            nc.sync.dma_start(
                wk[:], WK.ap().rearrange("(c p) m -> p c m", p=128))
            nc.sync.dma_start(
                wv[:], WV.ap().rearrange("(c p) m -> p c m", p=128))
            nc.sync.dma_start(
                wo[:], WO.ap().rearrange("(c p) m -> p c m", p=128))
            for nq in range(n_q):
                nc.sync.dma_start(
                    xt[:, nq], X[nq * 512:(nq + 1) * 512, :], transpose=True)

            with (
                tc.tile_pool(name="ppp", bufs=2, space="PSUM") as ppp,
                tc.tile_pool(name="stpp", bufs=2, space="PSUM") as stpp,
                tc.tile_pool(name="avp", bufs=2, space="PSUM") as avp,
                tc.tile_pool(name="work", bufs=4) as work,
                tc.tile_pool(name="osbp", bufs=8) as osbp,
            ):
                def gen_proj(nq):
                    """Projection of seq chunk nq; yields per PE quantum.
                    Order Q-pc, K-pc, V-st interleaved so the first
                    attention unit's inputs land earliest."""
                    for pc in range(n_pc):
                        for w, dst, cpy in ((wq, qt, "act"), (wk, kt, "dve")):
                            ps = ppp.tile([128, 512], F32, tag="pp",
                                          name=f"psp{nq}_{pc}")
                            for dc in range(n_dc):
                                nc.tensor.matmul(
                                    ps[:], w[:, dc, pc * 128:(pc + 1) * 128],
                                    xt[:, nq, dc, :],
                                    start=(dc == 0), stop=(dc == n_dc - 1))
                                yield
                            qs = slice(nq * 512, (nq + 1) * 512)
                            if cpy == "act":
                                nc.scalar.copy(dst[pc][:, qs], ps[:])
                            else:
                                nc.vector.tensor_copy(dst[pc][:, qs], ps[:])
                            yield
                        st = 4 * nq + pc
                        ps = ppp.tile([128, dl], F32, tag="pp",
                                      name=f"psv{nq}_{st}")
                        for dc in range(n_dc):
                            nc.tensor.matmul(
                                ps[:], xt[:, nq, dc, pc * 128:(pc + 1) * 128],
                                wv[:, dc, :],
                                start=(dc == 0), stop=(dc == n_dc - 1))
                            yield
                        nc.gpsimd.memset(vt[st][:], 1.0)
                        nc.vector.tensor_copy(
                            vt[st][:, :, 0:64],
                            ps[:].rearrange("p (h e) -> p h e", h=hl))
                        yield

                def gen_outproj(j, skip_last_pc=False):
                    """Output projection for seq chunk j. With skip_last_pc,
                    only head-pairs 0..n_pc-2 are accumulated and written to
                    OUT; the last pair goes to OUT2 via the finisher once
                    its normalize lands (summed on the host)."""
                    npc = n_pc - 1 if skip_last_pc else n_pc
                    for st in range(4 * j, 4 * j + 4):
                        for cc in range(n_cc):
                            ps = ppp.tile([128, 512], F32, tag="pp",
                                          name=f"pso{st}_{cc}")
                            for pc in range(npc):
                                nc.tensor.matmul(
                                    ps[:], ot[pc][:, st * 128:(st + 1) * 128],
                                    wo[:, pc, cc * 512:(cc + 1) * 512],
                                    start=(pc == 0), stop=(pc == npc - 1))
                                yield
                            osb = osbp.tile([128, 512], BF16, tag="osb",
                                            name=f"osb{st}_{cc}")
                            nc.vector.tensor_copy(osb[:], ps[:])
                            yield
                            # alternate rings so OUT writes don't back up
                            eng = nc.sync if (st + cc) % 2 == 0 else nc.scalar
                            eng.dma_start(
                                OUT[st * 128:(st + 1) * 128,
                                    cc * 512:(cc + 1) * 512],
                                osb[:])
                            yield

                def attn_unit(j, pc, fillers):
                    js = slice(j * 512, (j + 1) * 512)
                    n_i = min(4 * j + 4, n_k)
                    av = [avp.tile([VW, 512], F32, tag="av",
                                   name=f"av{j}_{pc}_{h}") for h in (0, 1)]

                    def emit_av(pet, prs, pi):
                        for h in (0, 1):
                            nc.tensor.matmul(
                                av[h][:, prs:512], vt[pi][:, 2 * pc + h, :],
                                pet[:, h, prs:512],
                                start=(pi == 0), stop=(pi == n_i - 1))

                    # AV pairs are emitted one block behind their exp (after
                    # the NEXT block's QK), so in PE priority order there is
                    # ~2 matmuls of ready work between an exp and the AV
                    # that consumes it — absorbs the exp latency the AVs
                    # were measured stalling on (~0.6us each).
                    prev = None
                    for i in range(n_i):
                        r = i - 4 * j
                        rs = max(r, 0) * 128   # fully-masked leading cols
                        stp = stpp.tile([128, 2, 512], F32, tag="stp",
                                        name=f"stp{j}_{pc}_{i}")
                        for h in (0, 1):
                            hs = slice(64 * h, 64 * h + 64)
                            nc.tensor.matmul(
                                stp[:, h, rs:512],
                                kt[pc][hs, i * 128:(i + 1) * 128],
                                qt[pc][hs, j * 512 + rs:(j + 1) * 512],
                                start=True, stop=True,
                                tile_position=(64 * h, 0))
                        if r >= 0:
                            nc.vector.tensor_add(
                                stp[:, :, rs:rs + 128], stp[:, :, rs:rs + 128],
                                cmask2[:])
                        et = work.tile([128, 2, 512], BF16, tag="et", bufs=6,
                                       name=f"et{j}_{pc}_{i}")
                        nc.scalar.activation(
                            et[:, :, rs:512], stp[:, :, rs:512], EXPF,
                            scale=0.125)
                        for flr, rate in fillers:
                            flr.pump(rate / 2)
                        if prev is not None:
                            emit_av(*prev)
                        prev = (et, rs, i)
                        for flr, rate in fillers:
                            flr.pump(rate / 2)
                    emit_av(*prev)

                    # normalize: denominators live in av row 64. Per-head
                    # independent chains: copy O'+denom to SBUF, DMA the
                    # denom row to partition 0, approx-reciprocal, gpsimd
                    # broadcast, DVE multiply (h1 DMA-shifts to rows 64-127).
                    for h in (0, 1):
                        orw = work.tile([VW, 512], F32, tag="orw", bufs=4,
                                        name=f"orw{j}_{pc}_{h}")
                        nc.vector.tensor_copy(orw[:], av[h][:])
                        dgp = work.tile([1, 512], F32, tag=f"dg{h}", bufs=3,
                                        name=f"dg{j}_{pc}_{h}")
                        nc.sync.dma_start(dgp[:], orw[64:65, :])
                        rgp = work.tile([1, 512], F32, tag=f"rg{h}", bufs=3,
                                        name=f"rg{j}_{pc}_{h}")
                        nc.vector.reciprocal_approx_fast(rgp[:], dgp[:])
                        bc = work.tile([64, 512], F32, tag=f"bc{h}", bufs=3,
                                       name=f"bc{j}_{pc}_{h}")
                        nc.gpsimd.partition_broadcast(bc[:], rgp[:])
                        if h == 0:
                            nc.vector.tensor_mul(
                                ot[pc][0:64, js], orw[0:64, :], bc[:])
                        else:
                            sc = work.tile([64, 512], BF16, tag="sc", bufs=3,
                                           name=f"sc{j}_{pc}")
                            nc.vector.tensor_mul(sc[:], orw[0:64, :], bc[:])
                            nc.sync.dma_start(ot[pc][64:128, js], sc[:])

                # ---- ramp: projections for chunk 0 ----
                for _ in gen_proj(0):
                    pass

                # ---- pipelined stages ----
                for j in range(n_q):
                    if j < n_q - 1:
                        filler = _Filler([gen_proj(j + 1)])
                        rate = {0: 7.0, 1: 3.6, 2: 2.4}[j]
                    else:
                        filler = _Filler([gen_outproj(0), gen_outproj(1),
                                          gen_outproj(2)])
                        rate = 2.0
                    for pc in range(n_pc):
                        fillers = [(filler, rate)]
                        if j == n_q - 1 and pc == n_pc - 1:
                            part1 = _Filler([gen_outproj(3, skip_last_pc=True)])
                            fillers.append((part1, 2.0))
                        attn_unit(j, pc, fillers)
                        if j == n_q - 1 and pc == n_pc - 1:
                            part1.drain()
                    filler.drain()

                # ---- finisher: last head-pair x chunk 3 -> OUT2 ----
                for st in range(4 * (n_q - 1), 4 * n_q):
                    for cc in range(n_cc):
                        psb = ppp.tile([128, 512], F32, tag="pp",
                                       name=f"psb{st}_{cc}")
                        nc.tensor.matmul(
                            psb[:], ot[n_pc - 1][:, st * 128:(st + 1) * 128],
                            wo[:, n_pc - 1, cc * 512:(cc + 1) * 512],
                            start=True, stop=True)
                        osb = osbp.tile([128, 512], BF16, tag="osb",
                                        name=f"osb2{st}_{cc}")
                        nc.vector.tensor_copy(osb[:], psb[:])
                        eng = nc.sync if (st + cc) % 2 == 0 else nc.scalar
                        eng.dma_start(
                            OUT2[(st - 4 * (n_q - 1)) * 128:
                                 (st - 4 * (n_q - 1) + 1) * 128,
                                 cc * 512:(cc + 1) * 512],
                            osb[:])

    nc.compile()
    return nc


_NC_CACHE = {}


def _get_program():
    key = (S, D, HL)
    if key not in _NC_CACHE:
        _NC_CACHE[key] = build_program()
    return _NC_CACHE[key]


def _bf16(a):
    return np.ascontiguousarray(a.astype(ml_dtypes.bfloat16))


def make_in_maps(X, Wq, Wk, Wv, Wo):
    in_maps = []
    for c in range(8):
        b, hg = c // 2, c % 2
        cs = slice(hg * DL, hg * DL + DL)
        in_maps.append({
            "X": _bf16(X[b]),
            "WQ": _bf16(Wq[:, cs]),
            "WK": _bf16(Wk[:, cs]),
            "WV": _bf16(Wv[:, cs]),
            "WO": _bf16(Wo[cs, :]),
        })
    return in_maps


def gather_out(results):
    out = np.empty((B, S, D), dtype=np.float32)
    for b in range(B):
        out[b] = (results[2 * b]["OUT"].astype(np.float32)
                  + results[2 * b + 1]["OUT"].astype(np.float32))
        out[b, S - 512:] += (results[2 * b]["OUT2"].astype(np.float32)
                             + results[2 * b + 1]["OUT2"].astype(np.float32))
    return out


def kernel(X, Wq, Wk, Wv, Wo):
    X = np.asarray(X, dtype=np.float32)
    Wq = np.asarray(Wq, dtype=np.float32)
    Wk = np.asarray(Wk, dtype=np.float32)
    Wv = np.asarray(Wv, dtype=np.float32)
    Wo = np.asarray(Wo, dtype=np.float32)

    nc = _get_program()
    in_maps = make_in_maps(X, Wq, Wk, Wv, Wo)
    res = run_bass_kernel_spmd(nc, in_maps, list(range(8)), trace=False)
    return gather_out(res.results)


if __name__ == "__main__":
    rng = np.random.default_rng(0)
    scale = 1.0 / np.sqrt(D)
    inputs = {
        "X": rng.standard_normal((B, S, D), dtype=np.float32),
        "Wq": rng.standard_normal((D, D), dtype=np.float32) * scale,
        "Wk": rng.standard_normal((D, D), dtype=np.float32) * scale,
        "Wv": rng.standard_normal((D, D), dtype=np.float32) * scale,
        "Wo": rng.standard_normal((D, D), dtype=np.float32) * scale,
    }
    out = kernel(**inputs)
    print("kernel output shape:", out.shape)
